# revision 66
# baseline (speedup 1.0000x reference)
"""Trainium2 Bass kernel for banded local attention.

Reference computation (B=2, S=2048, D=512, H=8, dh=64, local_range=7):
  q = hs @ Wq, k = hs @ Wk (per-head slices)
  scores = q k^T / sqrt(dh); w = softmax(scores) * band; w /= sum(w) + 1e-6
  ctx = w @ hs                                  -> [B, H, S, D]

Since w is re-normalized over the band, softmax(scores)*band/sum ==
band-limited softmax up to the tiny 1e-6*Z correction (~1e-4 relative),
so we only ever compute the 15-diagonal band of scores.

Sharding: sequence-sharded. Core c handles batch c//4, rows
[512*(c%4), 512*(c%4)+512), ALL 8 heads. Each core loads only its
~526-row hs window (plus replicated Wq/Wk), so per-core input DMA is
~2.3 MB instead of the ~4.3 MB a head-sharded split would need.

Row tiling: R=114-row output tiles have a j-window of R+14 = 128 rows,
which exactly fits one 128-partition matmul contraction. Scores are
computed directly TRANSPOSED (psum[j, i] via lhsT=kT, rhs=q), so E^T
is available for the context matmul without any PE transpose, and the
context is ONE 512-col matmul per (tile, head). The additive band mask
is injected into the scores psum accumulation by an identity matmul,
so ACT's exp directly yields masked E; the band rowsum is a
partition_all_reduce on the otherwise-idle Pool engine, and E is
normalized in SBUF (approx-reciprocal + multiply on DVE) before the
context matmul, so context psums hold final values and evict as plain
2-head copies split between ACT and DVE.

Per-head 64-lane score contractions are realized as full 128-lane
matmuls against zero-padded q planes (qTz): the PE faults when
64-contraction matmuls at different tile positions run back-to-back,
and the tile scheduler reorders instructions, so sub-tile matmuls are
avoided entirely.

Output is written bf16 (halves the dominant output DMA) and upcast to
f32 on the host.
"""

import os
import numpy as np
import ml_dtypes

DBG = set(os.environ.get("K_DBG", "").split(","))

BF = ml_dtypes.bfloat16
S, D, H, DH = 2048, 512, 8, 64
NCORES = 8
CR = 512           # rows per core
NT = 5             # row tiles per core (4*114 + 56)
WIN = 526          # hs window rows (512 + 2*7)
WINPAD = 584       # padded window so tile slots are uniform (114*4+128)
R_LIST = [114, 114, 114, 114, 56]
W_LIST = [128, 128, 128, 128, 70]

TRACE = False
LAST_RESULTS = None

_NC_CACHE = {}


def _build_nc():
    import concourse.bacc as bacc
    import concourse.mybir as mybir
    import concourse.tile as tile
    from concourse import bass_isa

    f32 = mybir.dt.float32
    bf16 = mybir.dt.bfloat16
    AF = mybir.ActivationFunctionType
    MUL = mybir.AluOpType.mult
    DIV = mybir.AluOpType.divide

    nc = bacc.Bacc("TRN2", target_bir_lowering=False, debug=False, num_devices=NCORES)

    # lhsT-packed projections: [d%128, d//128, hd]; q cols 0:512, k 512:1024
    wqk = nc.dram_tensor("wqk", [128, 4, 1024], bf16, kind="ExternalInput").ap()
    # transposed hs window: hst[p, dc, s] = hs_pad[s, 128*dc+p]
    hst = nc.dram_tensor("hst", [128, 4, WIN], bf16, kind="ExternalInput").ap()
    # banded hs window slots: hsw[p, t, d] = hs_pad[114t+p, d]
    hsw = nc.dram_tensor("hsw", [128, 5, 512], bf16, kind="ExternalInput").ap()
    # additive band mask (0 in band / -10000 outside) in [j, i] layout, per tile
    maskt = nc.dram_tensor("maskt", [128, 5, 114], bf16, kind="ExternalInput").ap()
    # 128x128 identity for injecting the mask into scores psums
    ident = nc.dram_tensor("ident", [128, 128], bf16, kind="ExternalInput").ap()
    out = nc.dram_tensor("out", [H, CR, D], bf16, kind="ExternalOutput").ap()
    out_r = out.rearrange("h s d -> s h d")

    with tile.TileContext(nc) as tc:
        with (
            tc.tile_pool(name="const", bufs=1) as cpool,
            tc.tile_pool(name="ework", bufs=8) as epool,
            tc.tile_pool(name="owork", bufs=4) as opool,
            tc.tile_pool(name="psc", bufs=2, space="PSUM") as pscp,
            tc.tile_pool(name="pbig", bufs=3, space="PSUM") as pbig,
        ):
            # ---- input loads (chunked so the first proj matmul starts early)
            wqk_sb = cpool.tile([128, 4, 1024], bf16)
            hst_sb = cpool.tile([128, 4, WIN], bf16)
            for dc in range(4):
                nc.sync.dma_start(out=wqk_sb[:, dc], in_=wqk[:, dc])
                nc.sync.dma_start(out=hst_sb[:, dc], in_=hst[:, dc])
            hsw_sb = cpool.tile([128, 5, 512], bf16)
            nc.sync.dma_start(out=hsw_sb, in_=hsw)
            mask_sb = cpool.tile([128, 5, 114], bf16)
            nc.sync.dma_start(out=mask_sb, in_=maskt)
            id_sb = cpool.tile([128, 128], bf16)
            nc.sync.dma_start(out=id_sb, in_=ident)
            # mask replicated over the 4 heads of a score group, so the
            # identity-matmul mask inject covers a whole group in one shot
            maskrep = cpool.tile([128, 5, 4, 114], bf16)
            nc.vector.tensor_copy(
                maskrep,
                mask_sb[:].unsqueeze(2).broadcast_to((128, 5, 4, 114)),
            )

            # ---- projections ----
            # The PE faults when 64-contraction matmuls at different tile
            # positions get scheduled back-to-back, so every matmul here
            # uses the full 128-partition contraction. The per-head scores
            # contraction (64 of the 128 hd lanes) is realized by storing q
            # zero-padded per head: qTz plane `head` holds q values on
            # partitions [64*(head%2), +64) and zeros elsewhere, so a dense
            # 128-lane dot against kT yields exactly that head's scores.
            # The zeroing is folded into the psum eviction via a 0/1
            # per-partition scale.
            qTz = cpool.tile([128, 8, 512], bf16)
            kT = cpool.tile([128, 4, WIN], bf16)
            zm = cpool.tile([128, 2], f32)
            nc.gpsimd.memset(zm[0:64, 0:1], 1.0)
            nc.gpsimd.memset(zm[64:128, 0:1], 0.0)
            nc.gpsimd.memset(zm[0:64, 1:2], 0.0)
            nc.gpsimd.memset(zm[64:128, 1:2], 1.0)

            def emit_proj_q(hdt):
                pq = pbig.tile([128, 2, 512], f32, tag="pbig")
                pqv = pq[:, 0]
                for dc in range(4):
                    nc.tensor.matmul(
                        pqv, wqk_sb[:, dc, 128 * hdt:128 * hdt + 128],
                        hst_sb[:, dc, 7:519], start=(dc == 0), stop=(dc == 3),
                    )
                nc.scalar.activation(qTz[:, 2 * hdt], pqv, AF.Copy,
                                     scale=zm[:, 0:1])
                nc.scalar.activation(qTz[:, 2 * hdt + 1], pqv, AF.Copy,
                                     scale=zm[:, 1:2])

            def emit_proj_k(hdt):
                pk = pbig.tile([128, 2, 512], f32, tag="pbig")
                for half in range(2):
                    pkv = pk[:, half, 0:263]
                    cs = 263 * half
                    for dc in range(4):
                        nc.tensor.matmul(
                            pkv, wqk_sb[:, dc, 512 + 128 * hdt:512 + 128 * hdt + 128],
                            hst_sb[:, dc, cs:cs + 263],
                            start=(dc == 0), stop=(dc == 3),
                        )
                # one strided eviction covers both 263-col psum halves
                nc.vector.tensor_copy(
                    kT[:, hdt].rearrange("p (a b) -> p a b", a=2),
                    pk[:, :, 0:263],
                )

            # ---- banded attention per row tile, 4-head score groups ----
            # The additive band mask rides in the scores psum accumulation
            # (identity-matmul inject), so exp directly yields masked E.
            def emit_scores(t, g):
                R, W = R_LIST[t], W_LIST[t]
                psc = pscp.tile([128, 456], f32, tag="psc")
                nc.tensor.matmul(
                    psc[0:W, :], id_sb[:, 0:W], maskrep[:, t],
                    start=True, stop=False,
                )
                for h4 in range(4):
                    head = 4 * g + h4
                    hdt = head // 2
                    nc.tensor.matmul(
                        psc[0:W, 114 * h4:114 * h4 + R],
                        kT[0:128, hdt, 114 * t:114 * t + W],
                        qTz[0:128, head, 114 * t:114 * t + R],
                        start=False, stop=(h4 == 3),
                    )
                return psc

            def emit_exp(t, g, psc):
                R, W = R_LIST[t], W_LIST[t]
                # masked E straight from psum; fold 1/sqrt(dh) into the scale
                Em = epool.tile([128, 4, 114], bf16, tag="Em")
                pv = psc[:].rearrange("p (h r) -> p h r", h=4)
                nc.scalar.activation(Em[0:W, :, 0:R], pv[0:W, :, 0:R], AF.Exp,
                                     scale=1.0 / (DH ** 0.5))
                # band rowsum across partitions (Pool), normalize (DVE)
                rs = epool.tile([128, 4, 114], f32, tag="rs")
                nc.gpsimd.partition_all_reduce(
                    rs[0:W, :, 0:R], Em[0:W, :, 0:R], channels=W,
                    reduce_op=bass_isa.ReduceOp.add,
                )
                rsr = epool.tile([128, 4, 114], f32, tag="rsr")
                nc.vector.reciprocal_approx_fast(rsr[0:W, :, 0:R], rs[0:W, :, 0:R])
                En = epool.tile([128, 4, 114], bf16, tag="En")
                nc.vector.tensor_tensor(out=En[0:W, :, 0:R], in0=Em[0:W, :, 0:R],
                                        in1=rsr[0:W, :, 0:R], op=MUL)
                return En

            # eviction engine per (tile, head-pair): ~12 ACT / 8 DVE
            EVICT_ENG = {(t, p): (1 if (2 * t + p) % 5 in (1, 3) else 0)
                         for t in range(NT) for p in range(4)}

            ctx_state = {}

            def emit_ctx_pair(t, pair, En):
                R, W = R_LIST[t], W_LIST[t]
                if pair == 0:
                    o_t = opool.tile([128, 8, 512], bf16, tag="o")
                    ctx_state[t] = o_t
                o_t = ctx_state[t]
                pc = pbig.tile([128, 2, 512], f32, tag="pbig")
                for j in range(2):
                    head = 2 * pair + j
                    nc.tensor.matmul(
                        pc[0:R, j], En[0:W, head % 4, 0:R], hsw_sb[0:W, t],
                        start=True, stop=True,
                    )
                src = pc[0:R]
                dst = o_t[0:R, 2 * pair:2 * pair + 2]
                if EVICT_ENG[t, pair] == 0:
                    nc.scalar.activation(dst, src, AF.Copy)
                else:
                    nc.vector.tensor_copy(dst, src)
                if pair == 1:
                    nc.sync.dma_start(out=out_r[114 * t:114 * t + R, 0:4],
                                      in_=o_t[0:R, 0:4])
                if pair == 3:
                    nc.sync.dma_start(out=out_r[114 * t:114 * t + R, 4:8],
                                      in_=ctx_state.pop(t)[0:R, 4:8])

            # ---- emission: software-pipeline tiles so the exp/mask/rowsum
            # chain of tile t hides behind the scores matmuls of tiles t+1/t+2
            def emit_tile_scores(t):
                return (emit_exp(t, 0, emit_scores(t, 0)),
                        emit_exp(t, 1, emit_scores(t, 1)))

            emit_proj_q(0); emit_proj_k(0)
            emit_proj_q(1); emit_proj_k(1)
            e00 = emit_exp(0, 0, emit_scores(0, 0))
            emit_proj_q(2); emit_proj_k(2)
            e10 = emit_exp(1, 0, emit_scores(1, 0))
            emit_proj_q(3); emit_proj_k(3)
            Ens = {0: (e00, emit_exp(0, 1, emit_scores(0, 1))),
                   1: (e10, emit_exp(1, 1, emit_scores(1, 1)))}
            for t in range(NT):
                if t + 2 < NT:
                    Ens[t + 2] = emit_tile_scores(t + 2)
                En = Ens.pop(t)
                for p in range(4):
                    emit_ctx_pair(t, p, En[0 if p < 2 else 1])

    nc.compile()
    return nc


def _get_nc():
    if "nc" not in _NC_CACHE:
        _NC_CACHE["nc"] = _build_nc()
    return _NC_CACHE["nc"]


def make_in_maps(hidden_states, Wq, Wk):
    hs_bf = np.asarray(hidden_states).astype(BF)
    wq_bf = np.asarray(Wq).astype(BF)
    wk_bf = np.asarray(Wk).astype(BF)

    # [128, 4, 1024] lhsT packing of Wq|Wk
    wqk_host = np.ascontiguousarray(
        np.concatenate(
            [wq_bf.reshape(4, 128, D), wk_bf.reshape(4, 128, D)], axis=2
        ).transpose(1, 0, 2)
    )

    jj = np.arange(128)[:, None]
    ii = np.arange(114)[None, :]

    in_maps = []
    for c in range(NCORES):
        b, quad = c // 4, c % 4
        gs0 = CR * quad
        lo = gs0 - 7
        pad = np.zeros((WINPAD, D), BF)
        s0, s1 = max(lo, 0), min(lo + WINPAD, S)
        pad[s0 - lo:s1 - lo] = hs_bf[b, s0:s1]

        hst_host = np.ascontiguousarray(
            pad[:WIN].T.reshape(4, 128, WIN).transpose(1, 0, 2)
        )
        hsw_host = np.empty((128, 5, 512), BF)
        for t in range(5):
            hsw_host[:, t] = pad[114 * t:114 * t + 128]
        mask_host = np.zeros((128, 5, 114), np.float32)
        for t in range(5):
            gj = lo + 114 * t + jj
            band = ((jj - ii >= 0) & (jj - ii <= 14)
                    & (gj >= 0) & (gj < S) & (114 * t + ii < CR))
            mask_host[:, t] = np.where(band, 0.0, -10000.0)

        in_maps.append({
            "wqk": wqk_host,
            "hst": hst_host,
            "hsw": np.ascontiguousarray(hsw_host),
            "maskt": np.ascontiguousarray(mask_host.astype(BF)),
            "ident": np.eye(128, dtype=BF),
        })
    return in_maps


def kernel(hidden_states, Wq, Wk):
    global LAST_RESULTS
    from concourse import bass_utils

    B = hidden_states.shape[0]
    in_maps = make_in_maps(hidden_states, Wq, Wk)

    nc = _get_nc()
    res = bass_utils.run_bass_kernel_spmd(
        nc, in_maps, core_ids=list(range(NCORES)), trace=TRACE,
    )
    LAST_RESULTS = res

    out = np.empty((B, H, S, D), np.float32)
    for c in range(NCORES):
        b, quad = c // 4, c % 4
        out[b, :, CR * quad:CR * quad + CR, :] = \
            np.asarray(res.results[c]["out"]).astype(np.float32)
    return out
